# revision 21
# baseline (speedup 1.0000x reference)
"""DIN sparse-attention kernel for 8 trn2 NeuronCores.

Strategy: data-parallel over batch (128 rows/core); embedding tables and MLP
weights replicated. Per core: two-level indirect-DMA gather (item rows, then
cate ids -> cate rows), PE transposes into a d-major bf16 layout, attention
MLP (512->80->40->1) as stationary-weight matmuls streaming bt columns,
masked softmax over T, weighted sum back through PE, then the folded dense
tail (BN1/BN2/dense collapsed into fc1 on the host).
"""
import sys

for _p in ("/opt/trn_rl_repo", "/root/.axon_site/_ro/trn_rl_repo"):
    if _p not in sys.path:
        sys.path.insert(0, _p)

import numpy as np
import ml_dtypes

import concourse.bass as bass
import concourse.tile as tile
from concourse import mybir
from concourse.bass import IndirectOffsetOnAxis
from concourse.bass_utils import run_bass_kernel_spmd
from concourse.masks import make_identity
from concourse.vector_clock import ScopedClock

f32 = mybir.dt.float32
bf16 = mybir.dt.bfloat16
i32 = mybir.dt.int32
AF = mybir.ActivationFunctionType
OP = mybir.AluOpType
AX = mybir.AxisListType

B, T, H = 1024, 200, 64
D = 2 * H              # 128
ITEM_N, CATE_N = 100000, 1000
NCORES = 8
BLOC = B // NCORES     # 128
C = 16                 # batch chunk per pipeline stage
NCHUNK = BLOC // C     # 8
TP = 256               # padded hist cols
PADC = float((-2.0**32 + 1) / np.sqrt(np.float32(D)))
BN_EPS = 1e-3


# ---------------------------------------------------------------- tile patch
def _patched_drain_and_barrier(self, tick_clock, wait_clock):
    # walrus in this container rejects instructions carrying several sem
    # waits; put the end-of-kernel drain waits on nop instructions instead.
    nop1 = self.nc.sync.nop(nofuse=True)
    wait_clock.add_sem_waits(nop1.ins, ScopedClock({None: tick_clock.global_clock}))
    si = nop1.ins.sync_info
    if si is not None and len(si.on_wait) > 1:
        waits = list(si.on_wait)
        si.on_wait.clear()
        si.on_wait.append(waits[0])
        for w in waits[1:]:
            nop = self.nc.sync.nop(nofuse=True)
            si2 = nop.ins.sync_info
            if si2 is None:
                nop.ins.sync_info = mybir.SyncInfo(on_wait=[w], on_update=[])
            else:
                si2.on_wait.append(w)
    self.nc.sync.drain()
    self.nc.all_engine_barrier()
    assert self.sems is not None
    popped = self.nc._tile_sem_poison_stack.pop()
    assert popped is self._sem_poison
    self.nc.clear_and_free_semaphores(list(self.sems.allocated().values()))
    self.nc.all_engine_barrier()


tile.TileContext._drain_and_barrier = _patched_drain_and_barrier


def _fix_excess_waits(nc):
    ctr = 0
    for f in nc.m.functions:
        for b in f.blocks:
            new_list = []
            for inst in b.instructions:
                si = inst.sync_info
                waits = list(si.on_wait) if (si is not None and si.on_wait) else []
                if len(waits) > 1:
                    si.on_wait.clear()
                    si.on_wait.append(waits[0])
                    for w in waits[1:]:
                        ctr += 1
                        new_list.append(mybir.InstNoOp(
                            name=f"waitfix-{ctr}", engine=inst.engine,
                            ins=[], outs=[], bass_nofuse=True,
                            sync_info=mybir.SyncInfo(on_wait=[w], on_update=[])))
                new_list.append(inst)
            b.instructions.clear()
            b.instructions.extend(new_list)


# ---------------------------------------------------------------- bass build
def build_nc():
    nc = bass.Bass()
    P = lambda n, s, d: nc.declare_dram_parameter(n, s, d, isOutput=False)

    hist_pad = P("hist_pad", [BLOC, TP], i32)
    item_q = P("item_q", [BLOC, 1], i32)
    sl_in = P("sl_in", [BLOC, 1], f32)
    cate_list = P("cate_list", [ITEM_N, 1], i32)
    item_table = P("item_table", [ITEM_N, H], f32)
    cate_table = P("cate_table", [CATE_N, H], f32)
    wk = P("wk", [D, 80], bf16)          # att_w1[k] - att_w1[q-k]
    wp = P("wp", [D, 80], bf16)          # att_w1[q*k]
    wqd = P("wqd", [D, 80], bf16)        # att_w1[q] + att_w1[q-k]
    b1 = P("b1", [80, 1], f32)
    w2 = P("w2", [80, 40], bf16)
    b2 = P("b2", [40, 1], f32)
    w3 = P("w3", [40, 1], bf16)          # att_w3 / sqrt(2H)
    wfold = P("wfold", [D, 80], f32)     # bn1+dense+bn2 folded into fc1 (u path)
    wie = P("wie", [D, 80], f32)         # bn2 folded fc1 (ie path)
    bfold = P("bfold", [80, 1], f32)
    fc2w = P("fc2w", [80, 40], f32)
    fc2b = P("fc2b", [40, 1], f32)
    fc3w = P("fc3w", [40, 1], f32)
    fc3b = P("fc3b", [1, 1], f32)
    y_out = nc.declare_dram_parameter("y_out", [BLOC, 1], f32, isOutput=True)
    a_scratch = nc.dram_tensor("a_scratch", [BLOC, 80], bf16)

    with tile.TileContext(nc) as tc:
        with tc.tile_pool(name="const", bufs=1) as cpool, \
             tc.tile_pool(name="glob", bufs=1) as gpool, \
             tc.tile_pool(name="work", bufs=2) as wpool, \
             tc.tile_pool(name="psA", bufs=2, space="PSUM") as psA, \
             tc.tile_pool(name="psB", bufs=2, space="PSUM") as psB, \
             tc.tile_pool(name="psC", bufs=2, space="PSUM") as psC, \
             tc.tile_pool(name="psG", bufs=1, space="PSUM") as psG:

            # ---------------- constants / weights in SBUF
            def load_const(ap, shape, dt):
                t = cpool.tile(shape, dt, tag=f"c{ap.name}")
                nc.sync.dma_start(out=t[:], in_=ap[:])
                return t

            wk_s = load_const(wk, [D, 80], bf16)
            wp_s = load_const(wp, [D, 80], bf16)
            wqd_s = load_const(wqd, [D, 80], bf16)
            b1_s = load_const(b1, [80, 1], f32)
            w2_s = load_const(w2, [80, 40], bf16)
            b2_s = load_const(b2, [40, 1], f32)
            w3_s = load_const(w3, [40, 1], bf16)
            wfold_s = load_const(wfold, [D, 80], f32)
            wie_s = load_const(wie, [D, 80], f32)
            bfold_s = load_const(bfold, [80, 1], f32)
            fc2w_s = load_const(fc2w, [80, 40], f32)
            fc2b_s = load_const(fc2b, [40, 1], f32)
            fc3w_s = load_const(fc3w, [40, 1], f32)
            fc3b_s = load_const(fc3b, [1, 1], f32)

            ident = cpool.tile([128, 128], f32, tag="ident")
            make_identity(nc, ident[:])
            ones200 = cpool.tile([1, 200], bf16, tag="ones200")
            nc.vector.memset(ones200[:], 1.0)
            iota_i = cpool.tile([128, 200], i32, tag="iotai")
            nc.gpsimd.iota(iota_i[:], pattern=[[1, 200]], base=0, channel_multiplier=0)
            iota_t = cpool.tile([128, 200], f32, tag="iota")
            nc.vector.tensor_copy(out=iota_t[:], in_=iota_i[:])
            padt = cpool.tile([128, 200], f32, tag="padt")
            nc.vector.memset(padt[:], PADC)

            # ---------------- query embeddings ie -> ieT (d-major)
            qid = gpool.tile([128, 1], i32, tag="qid")
            nc.sync.dma_start(out=qid[:], in_=item_q[:])
            qcid = gpool.tile([128, 1], i32, tag="qcid")
            qe = gpool.tile([128, 2, H], f32, tag="qe")
            nc.gpsimd.indirect_dma_start(
                out=qe[:, 0, :], out_offset=None, in_=item_table[:],
                in_offset=IndirectOffsetOnAxis(ap=qid[:], axis=0))
            nc.gpsimd.indirect_dma_start(
                out=qcid[:], out_offset=None, in_=cate_list[:],
                in_offset=IndirectOffsetOnAxis(ap=qid[:], axis=0))
            nc.gpsimd.indirect_dma_start(
                out=qe[:, 1, :], out_offset=None, in_=cate_table[:],
                in_offset=IndirectOffsetOnAxis(ap=qcid[:], axis=0))
            pT0 = psA.tile([128, 400], f32, space="PSUM", tag="pT")
            nc.tensor.matmul(out=pT0[0:H, 0:128], lhsT=qe[:, 0, :], rhs=ident[:],
                             start=True, stop=True, skip_group_check=True)
            nc.tensor.matmul(out=pT0[H:D, 0:128], lhsT=qe[:, 1, :], rhs=ident[:],
                             start=True, stop=True, skip_group_check=True)
            ieT_b = gpool.tile([D, BLOC], bf16, tag="ieTb")
            ieT_f = gpool.tile([D, BLOC], f32, tag="ieTf")
            nc.vector.tensor_copy(out=ieT_b[:], in_=pT0[:, 0:128])
            nc.scalar.copy(out=ieT_f[:], in_=pT0[:, 0:128])

            # A = q @ (Wq + Wd)  -> flattened to partition 0 for K=1 lhsT use
            pA = psB.tile([128, 400], f32, space="PSUM", tag="pm1")
            nc.tensor.matmul(out=pA[0:BLOC, 0:80], lhsT=ieT_b[:], rhs=wqd_s[:],
                             start=True, stop=True, skip_group_check=True)
            A_sb = gpool.tile([128, 80], bf16, tag="Asb")
            nc.vector.tensor_copy(out=A_sb[:], in_=pA[0:BLOC, 0:80])
            A_flat = gpool.tile([1, BLOC * 80], bf16, tag="Aflat")
            nc.sync.dma_start(out=a_scratch[:], in_=A_sb[:])
            nc.sync.dma_start(out=A_flat[:],
                              in_=a_scratch[:].rearrange("b a -> (b a)").rearrange("(o f) -> o f", o=1))

            # ---------------- global PSUM accumulators
            sc_ps = psG.tile([128, 256], f32, space="PSUM", tag="scps")
            nc.vector.memset(sc_ps[:], 0.0)
            attT = gpool.tile([D, BLOC], f32, tag="attT")

            y_all = gpool.tile([1, BLOC], f32, tag="yall")

            # ---------------- main chunk loop
            for c in range(NCHUNK):
                b0 = c * C
                ids = wpool.tile([128, C, 2], i32, tag="ids")
                nc.sync.dma_start(
                    out=ids[:],
                    in_=hist_pad[:].rearrange("b (h t) -> t b h", h=2)[:, b0:b0 + C, :])
                cid = wpool.tile([128, 2 * C], i32, tag="cid")
                he_it = wpool.tile([128, 2 * C, H], f32, tag="heit")
                he_ct = wpool.tile([128, 2 * C, H], f32, tag="hect")
                idsf = ids[:].rearrange("t c h -> t (c h)")
                for g0 in range(0, 2 * C, 4):
                    nc.gpsimd.indirect_dma_start(
                        out=he_it[:, g0:g0 + 4, :], out_offset=None, in_=item_table[:],
                        in_offset=IndirectOffsetOnAxis(ap=idsf[:, g0:g0 + 4], axis=0))
                    nc.gpsimd.indirect_dma_start(
                        out=cid[:, g0:g0 + 4], out_offset=None, in_=cate_list[:],
                        in_offset=IndirectOffsetOnAxis(ap=idsf[:, g0:g0 + 4], axis=0))
                    nc.gpsimd.indirect_dma_start(
                        out=he_ct[:, g0:g0 + 4, :], out_offset=None, in_=cate_table[:],
                        in_offset=IndirectOffsetOnAxis(ap=cid[:, g0:g0 + 4], axis=0))

                # transposes: per b: psum cols [0:128 lo | 128:200 hi], item rows
                # 0:64, cate rows 64:128; pack a pair of b per psum tile.
                heT = wpool.tile([D, C * 200], bf16, tag="heT")
                for p in range(C // 2):
                    pT = psA.tile([128, 400], f32, space="PSUM", tag="pT")
                    for k in range(2):           # b within pair
                        j = 2 * p + k
                        co = 200 * k
                        nc.tensor.matmul(out=pT[0:H, co:co + 128],
                                         lhsT=he_it[:, 2 * j, :], rhs=ident[:],
                                         start=True, stop=True, skip_group_check=True)
                        nc.tensor.matmul(out=pT[H:D, co:co + 128],
                                         lhsT=he_ct[:, 2 * j, :], rhs=ident[:],
                                         start=True, stop=True, skip_group_check=True)
                        nc.tensor.matmul(out=pT[0:H, co + 128:co + 200],
                                         lhsT=he_it[:, 2 * j + 1, :],
                                         rhs=ident[:, 0:72], start=True, stop=True, skip_group_check=True)
                        nc.tensor.matmul(out=pT[H:D, co + 128:co + 200],
                                         lhsT=he_ct[:, 2 * j + 1, :],
                                         rhs=ident[:, 0:72], start=True, stop=True, skip_group_check=True)
                    if p % 2 == 0:
                        nc.vector.tensor_copy(out=heT[:, p * 400:(p + 1) * 400], in_=pT[:])
                    else:
                        nc.scalar.copy(out=heT[:, p * 400:(p + 1) * 400], in_=pT[:])

                # qheT = heT * q  (bf16, broadcast q along t)
                qheT = wpool.tile([D, C * 200], bf16, tag="qheT")
                nc.vector.tensor_tensor(
                    out=qheT[:].rearrange("d (c t) -> d c t", c=C),
                    in0=heT[:].rearrange("d (c t) -> d c t", c=C),
                    in1=ieT_b[:, b0:b0 + C].to_broadcast([D, C, 200]),
                    op=OP.mult)

                # MLP1 + sigmoid -> s1T
                s1T = wpool.tile([80, C * 200], bf16, tag="s1T")
                for p in range(C // 2):
                    pm1 = psB.tile([128, 400], f32, space="PSUM", tag="pm1")
                    cc = p * 400
                    nc.tensor.matmul(out=pm1[0:80, 0:400], lhsT=wk_s[:],
                                     rhs=heT[:, cc:cc + 400], start=True, stop=False, skip_group_check=True)
                    nc.tensor.matmul(out=pm1[0:80, 0:400], lhsT=wp_s[:],
                                     rhs=qheT[:, cc:cc + 400], start=False, stop=False, skip_group_check=True)
                    for k in range(2):
                        bg = b0 + 2 * p + k
                        nc.tensor.matmul(out=pm1[0:80, 200 * k:200 * k + 200],
                                         lhsT=A_flat[0:1, bg * 80:(bg + 1) * 80],
                                         rhs=ones200[:], start=False,
                                         stop=(k == 1), skip_group_check=True)
                    nc.scalar.activation(out=s1T[:, p * 400:(p + 1) * 400],
                                         in_=pm1[0:80, 0:400], func=AF.Sigmoid,
                                         bias=b1_s[:], scale=1.0)

                # MLP2 + sigmoid -> s2T
                s2T = wpool.tile([40, C * 200], bf16, tag="s2T")
                for p in range(C // 2):
                    pm2 = psC.tile([64, 400], f32, space="PSUM", tag="pm2")
                    nc.tensor.matmul(out=pm2[0:40, 0:400], lhsT=w2_s[:],
                                     rhs=s1T[:, p * 400:(p + 1) * 400],
                                     start=True, stop=True, skip_group_check=True)
                    nc.scalar.activation(out=s2T[:, p * 400:(p + 1) * 400],
                                         in_=pm2[0:40, 0:400], func=AF.Sigmoid,
                                         bias=b2_s[:], scale=1.0)

                # MLP3: scores into sc_ps [t' , b(lo) | 128+b(hi)]
                for j in range(C):
                    col = b0 + j
                    s2c = j * 200
                    nc.tensor.matmul(out=sc_ps[0:128, col:col + 1],
                                     lhsT=s2T[:, s2c:s2c + 128], rhs=w3_s[:],
                                     start=True, stop=True, skip_group_check=True)
                    nc.tensor.matmul(out=sc_ps[0:72, 128 + col:129 + col],
                                     lhsT=s2T[:, s2c + 128:s2c + 200], rhs=w3_s[:],
                                     start=True, stop=True, skip_group_check=True)

                # scores -> [b, t], mask, softmax
                sc_sb = wpool.tile([128, C], f32, tag="scsb")
                sc_sb2 = wpool.tile([128, C], f32, tag="scsb2")
                nc.vector.tensor_copy(out=sc_sb[:], in_=sc_ps[:, b0:b0 + C])
                nc.vector.tensor_copy(out=sc_sb2[0:72, :], in_=sc_ps[0:72, 128 + b0:128 + b0 + C])
                p_bt = psA.tile([128, 400], f32, space="PSUM", tag="pT")
                nc.tensor.matmul(out=p_bt[0:C, 0:128], lhsT=sc_sb[:],
                                 rhs=ident[:], start=True, stop=True, skip_group_check=True)
                nc.tensor.matmul(out=p_bt[0:C, 128:200], lhsT=sc_sb2[0:72, :],
                                 rhs=ident[0:72, 0:72], start=True, stop=True, skip_group_check=True)
                bt = wpool.tile([C, 200], f32, tag="bt")
                nc.vector.tensor_copy(out=bt[:], in_=p_bt[0:C, 0:200])
                slc = wpool.tile([C, 1], f32, tag="slc")
                nc.sync.dma_start(out=slc[:], in_=sl_in[b0:b0 + C, :])
                mge = wpool.tile([C, 200], i32, tag="mge")
                nc.vector.tensor_scalar(out=mge[:], in0=iota_t[0:C, :],
                                        scalar1=slc[:], scalar2=None,
                                        op0=OP.is_ge)
                nc.vector.copy_predicated(out=bt[:], mask=mge[:], data=padt[0:C, :])
                nmax = wpool.tile([C, 1], f32, tag="nmax")
                nc.vector.tensor_reduce(out=nmax[:], in_=bt[:], axis=AX.X,
                                        op=OP.max, negate=True)
                ex = wpool.tile([C, 200], f32, tag="ex")
                rsum = wpool.tile([C, 1], f32, tag="rsum")
                nc.scalar.activation(out=ex[:], in_=bt[:], func=AF.Exp,
                                     bias=nmax[:], scale=1.0, accum_out=rsum[:])
                rinv = wpool.tile([C, 1], f32, tag="rinv")
                nc.vector.reciprocal(rinv[:], rsum[:])
                wsm = wpool.tile([C, 200], f32, tag="wsm")
                nc.vector.tensor_scalar_mul(wsm[:], ex[:], rinv[:])

                # wT: [t', lo b | hi b]
                p_wT = psA.tile([128, 400], f32, space="PSUM", tag="pT")
                nc.tensor.matmul(out=p_wT[0:128, 0:C], lhsT=wsm[:, 0:128],
                                 rhs=ident[0:C, 0:C], start=True, stop=True, skip_group_check=True)
                nc.tensor.matmul(out=p_wT[0:72, C:2 * C], lhsT=wsm[:, 128:200],
                                 rhs=ident[0:C, 0:C], start=True, stop=True, skip_group_check=True)
                wT = wpool.tile([128, 2 * C], f32, tag="wT")
                nc.vector.tensor_copy(out=wT[:, 0:C], in_=p_wT[0:128, 0:C])
                nc.vector.tensor_copy(out=wT[0:72, C:2 * C], in_=p_wT[0:72, C:2 * C])

                # weighted sum: att^T[d, b] for this chunk, then park in SBUF
                att_ps = psB.tile([128, 400], f32, space="PSUM", tag="pm1")
                for j in range(C):
                    nc.tensor.matmul(out=att_ps[0:H, j:j + 1],
                                     lhsT=he_it[:, 2 * j, :], rhs=wT[:, j:j + 1],
                                     start=True, stop=False, skip_group_check=True)
                    nc.tensor.matmul(out=att_ps[0:H, j:j + 1],
                                     lhsT=he_it[0:72, 2 * j + 1, :],
                                     rhs=wT[0:72, C + j:C + j + 1],
                                     start=False, stop=True, skip_group_check=True)
                    nc.tensor.matmul(out=att_ps[H:D, j:j + 1],
                                     lhsT=he_ct[:, 2 * j, :], rhs=wT[:, j:j + 1],
                                     start=True, stop=False, skip_group_check=True)
                    nc.tensor.matmul(out=att_ps[H:D, j:j + 1],
                                     lhsT=he_ct[0:72, 2 * j + 1, :],
                                     rhs=wT[0:72, C + j:C + j + 1],
                                     start=False, stop=True, skip_group_check=True)
                nc.vector.tensor_copy(out=attT[:, b0:b0 + C], in_=att_ps[:, 0:C])

            # ---------------- dense tail
            pd1 = psB.tile([128, 400], f32, space="PSUM", tag="pm1")
            nc.tensor.matmul(out=pd1[0:80, 0:BLOC], lhsT=wfold_s[:], rhs=attT[:],
                             start=True, stop=False, skip_group_check=True)
            nc.tensor.matmul(out=pd1[0:80, 0:BLOC], lhsT=wie_s[:], rhs=ieT_f[:],
                             start=False, stop=True, skip_group_check=True)
            sd1 = gpool.tile([80, BLOC], f32, tag="sd1")
            nc.scalar.activation(out=sd1[:], in_=pd1[0:80, 0:BLOC], func=AF.Sigmoid,
                                 bias=bfold_s[:], scale=1.0)
            pd2 = psC.tile([64, 400], f32, space="PSUM", tag="pm2")
            nc.tensor.matmul(out=pd2[0:40, 0:BLOC], lhsT=fc2w_s[:], rhs=sd1[:],
                             start=True, stop=True, skip_group_check=True)
            sd2 = gpool.tile([40, BLOC], f32, tag="sd2")
            nc.scalar.activation(out=sd2[:], in_=pd2[0:40, 0:BLOC], func=AF.Sigmoid,
                                 bias=fc2b_s[:], scale=1.0)
            pd3 = psC.tile([64, 400], f32, space="PSUM", tag="pm2")
            nc.tensor.matmul(out=pd3[0:1, 0:BLOC], lhsT=fc3w_s[:], rhs=sd2[:],
                             start=True, stop=True, skip_group_check=True)
            nc.scalar.activation(out=y_all[:], in_=pd3[0:1, 0:BLOC], func=AF.Sigmoid,
                                 bias=fc3b_s[:], scale=1.0)
            nc.sync.dma_start(
                out=y_out[:].rearrange("b x -> (b x)").rearrange("(o f) -> o f", o=1),
                in_=y_all[:])

    return nc


_NC_CACHE = None
_LAST_RES = None


def _get_nc():
    global _NC_CACHE
    if _NC_CACHE is None:
        _NC_CACHE = build_nc()
    return _NC_CACHE


def _fold_weights(inp):
    g = lambda k: np.asarray(inp[k], np.float32)
    s = 1.0 / np.sqrt(1.0 + BN_EPS)
    att_w1 = g("att_w1")
    wq, wkk, wd, wpp = att_w1[0:D], att_w1[D:2 * D], att_w1[2 * D:3 * D], att_w1[3 * D:4 * D]
    ga = g("bn1_g") * s
    ba = g("bn1_b")
    g2 = g("bn2_g") * s
    c2 = g("bn2_b")
    g2a, c2a = g2[0:H], c2[0:H]
    g2b, c2b = g2[H:3 * H], c2[H:3 * H]
    dense_w, dense_b = g("dense_w"), g("dense_b")
    fc1_w, fc1_b = g("fc1_w"), g("fc1_b")
    u_bias = ba @ dense_w + dense_b
    wfold = (ga[:, None] * dense_w) @ (g2a[:, None] * fc1_w[0:H])
    wie = g2b[:, None] * fc1_w[H:3 * H]
    bfold = (u_bias * g2a) @ fc1_w[0:H] + c2b @ fc1_w[H:3 * H] + fc1_b
    bf = lambda x: np.ascontiguousarray(x, np.float32).astype(ml_dtypes.bfloat16)
    fl = lambda x: np.ascontiguousarray(x, np.float32)
    return {
        "wk": bf(wkk - wd), "wp": bf(wpp), "wqd": bf(wq + wd),
        "b1": fl(g("att_b1")[:, None]),
        "w2": bf(g("att_w2")), "b2": fl(g("att_b2")[:, None]),
        "w3": bf(g("att_w3") / np.sqrt(np.float32(D))),
        "wfold": fl(wfold), "wie": fl(wie), "bfold": fl(bfold[:, None]),
        "fc2w": fl(g("fc2_w")), "fc2b": fl(g("fc2_b")[:, None]),
        "fc3w": fl(g("fc3_w")), "fc3b": fl(g("fc3_b")[:, None]),
    }


def kernel(**inputs):
    nc = _get_nc()
    if not getattr(nc, "_waitfix_done", False):
        _fix_excess_waits(nc)
        nc._waitfix_done = True
    folded = _fold_weights(inputs)
    hist = np.asarray(inputs["hist"], np.int32)
    item = np.asarray(inputs["item"], np.int32)
    sl = np.asarray(inputs["sl"], np.int32)
    shared = {
        "cate_list": np.ascontiguousarray(np.asarray(inputs["cate_list"], np.int32)[:, None]),
        "item_table": np.ascontiguousarray(inputs["item_table"], dtype=np.float32),
        "cate_table": np.ascontiguousarray(inputs["cate_table"], dtype=np.float32),
        **folded,
    }
    hist_pad = np.zeros((B, TP), np.int32)
    hist_pad[:, 0:T] = hist
    in_maps = []
    for ci in range(NCORES):
        lo, hi = ci * BLOC, (ci + 1) * BLOC
        in_maps.append({
            "hist_pad": np.ascontiguousarray(hist_pad[lo:hi]),
            "item_q": np.ascontiguousarray(item[lo:hi, None]),
            "sl_in": np.ascontiguousarray(sl[lo:hi, None].astype(np.float32)),
            **shared,
        })
    import os
    trace = bool(os.environ.get("KERNEL_TRACE"))
    res = run_bass_kernel_spmd(nc, in_maps, list(range(NCORES)), trace=trace)
    global _LAST_RES
    _LAST_RES = res
    y = np.concatenate([res.results[i]["y_out"] for i in range(NCORES)], axis=0)
    return y.astype(np.float32)
